# revision 1
# baseline (speedup 1.0000x reference)
"""Cross-attention Trainium2 kernel (8 NeuronCores, data-parallel over batch).

Reference computation per batch element b:
    x_flat = x[b].reshape(C, N).T                 # [N, C], N = H*W = 4096
    q = x_flat @ Wq ; k = ctx @ Wk ; v = ctx @ Wv  # heads=8, dim_head=64
    attn = softmax(q k^T / 8) ; o = attn v
    out = (o @ Wo + bo).T.reshape(C, H, W) + x[b]

Device layout (everything transposed so the HW-native [C, N] layout of x is
used directly; S^T = k q^T kept with m (context tokens) on partitions so the
softmax denominator comes free from an extra ones-row in v'):

  qT  [inner, 512]  = Wq^T x[:, nc]   streamed per n-chunk
  kT  [inner, M]    = Wk^T ctx^T      persistent
  v'  [M, 8*65]     = (ctx Wv | 1)    per-head 65-wide blocks: 64 v-cols + ones
  S^T [m 128, n]    = kT_h^T qT_h     fp32r matmuls, K=64
  P^T = exp(S^T * 0.125)              ScalarE, bf16 out
  O'  [65, n]       = v'^T P^T        bf16 matmuls, K=128, accumulated over m;
                                      row 64 = softmax denominators
  out = Wo^T (O'/sums) + bo + x       bf16 matmuls + DVE epilogue

Software pipeline (emission order = Tile priority):
  step t = (nci, h):  QK+exp(t) ; AV(t-1) ; pair-normalize as pairs complete;
  out-projection per n-chunk as its 4 pairs complete; Qproj(nci+1) mid-chunk.
This keeps ScalarE (the bottleneck: 33.5M exps at 1 elem/cycle/lane) fed
continuously while PE fills its slack with projections.
"""

import sys

for _p in ("/opt/trn_rl_repo", "/opt/pypackages"):
    if _p not in sys.path:
        sys.path.append(_p)

import numpy as np

import concourse.bass as bass
import concourse.tile as tile
from concourse import bacc, mybir
from concourse.bass_utils import run_bass_kernel_spmd

B, C, HH, WW = 8, 512, 64, 64
N = HH * WW            # 4096 query tokens
M = 1024               # context tokens
CTX = 768              # context channels
HEADS, DH = 8, 64
INNER = HEADS * DH     # 512
SCALE = DH ** -0.5     # 0.125

F32 = mybir.dt.float32
F32R = mybir.dt.float32r
BF16 = mybir.dt.bfloat16
AF = mybir.ActivationFunctionType
ALU = mybir.AluOpType

NCI = N // 512         # 8 n-chunks of 512
MCI = M // 128         # 8 m-chunks of 128
CCH = C // 128         # 4
XCH = CTX // 128       # 6
# S^T staging groups of m-chunks: 2 rotating 3-bank PSUM tiles + 2 accum banks
SGROUPS = [(0, 3), (3, 6), (6, 8)]

_PROG = None


def _build(reps=1):
    nc = bacc.Bacc("TRN2", target_bir_lowering=False, debug=False, num_devices=8)

    x_d = nc.dram_tensor("x", [C, N], F32R, kind="ExternalInput")
    ctxt_d = nc.dram_tensor("ctxT", [CTX, M], F32R, kind="ExternalInput")
    wq_d = nc.dram_tensor("wq", [C, INNER], F32R, kind="ExternalInput")
    wk_d = nc.dram_tensor("wk", [CTX, INNER], F32R, kind="ExternalInput")
    wv_d = nc.dram_tensor("wv", [CTX, INNER], F32R, kind="ExternalInput")
    wo_d = nc.dram_tensor("wo", [INNER, C], F32, kind="ExternalInput")
    bo_d = nc.dram_tensor("bo", [C], F32, kind="ExternalInput")
    out_d = nc.dram_tensor("out", [C, N], F32, kind="ExternalOutput")
    # DRAM bounce for the reciprocal row-broadcast (SBUF APs can't have
    # zero partition step; DRAM APs can)
    rscr_d = nc.dram_tensor("rscr", [64, 512], BF16)

    with tile.TileContext(nc) as tc:
        with (
            tc.tile_pool(name="pers", bufs=1) as pers,
            tc.tile_pool(name="xs", bufs=4) as xs,
            tc.tile_pool(name="qp", bufs=6) as qp,
            tc.tile_pool(name="pp", bufs=5) as pp,
            tc.tile_pool(name="misc", bufs=2) as misc,
            tc.tile_pool(name="rrp", bufs=1) as rrp,
            tc.tile_pool(name="sump", bufs=4) as sump,
            tc.tile_pool(name="ost", bufs=2) as ostp,
            tc.tile_pool(name="outp", bufs=2) as outp,
            tc.tile_pool(name="xr", bufs=2) as xrp,
            tc.tile_pool(name="psS", bufs=2, space="PSUM") as psS,
            tc.tile_pool(name="psA", bufs=2, space="PSUM") as psA,
        ):
            for _rep in range(reps):
                # ---------------- persistent tiles + weight loads ----------------
                wq_sb = [pers.tile([128, INNER], F32R, tag=f"wq{c}", name=f"wq{c}")
                         for c in range(CCH)]
                wo_bf = [pers.tile([128, C], BF16, tag=f"wob{k}", name=f"wob{k}")
                         for k in range(CCH)]
                bo_sb = pers.tile([128, CCH], F32, tag="bo")
                ones_sb = pers.tile([128, 8], F32, tag="ones")
                nc.vector.memset(ones_sb[:], 1.0)
                kt_sb = [pers.tile([128, M], F32R, tag=f"kt{i}", name=f"kt{i}")
                         for i in range(CCH)]
                v_sb = [pers.tile([128, HEADS * (DH + 1)], F32R, tag=f"v{m}",
                                  name=f"v{m}") for m in range(MCI)]
                o_all = [pers.tile([128, N], BF16, tag=f"oa{k}", name=f"oa{k}")
                         for k in range(CCH)]

                wkr = wk_d.rearrange("(a p) i -> a p i", p=128)
                wvr = wv_d.rearrange("(a p) i -> a p i", p=128)
                wqr = wq_d.rearrange("(a p) i -> a p i", p=128)
                wor = wo_d.rearrange("(a p) i -> a p i", p=128)
                for c in range(CCH):
                    nc.sync.dma_start(out=wq_sb[c], in_=wqr[c])
                for c in range(CCH):
                    wos = outp.tile([128, C], F32, tag="out", name="wos",
                                    padded_shape=None)
                    nc.gpsimd.dma_start(out=wos, in_=wor[c])
                    nc.vector.tensor_copy(wo_bf[c], wos)
                nc.gpsimd.dma_start(out=bo_sb, in_=bo_d.rearrange("(a p) -> p a", p=128))

                xdr = x_d.rearrange("(a p) n -> a p n", p=128)
                ctxr = ctxt_d.rearrange("(a p) m -> a p m", p=128)

                def qproj_load(nci):
                    """stream x[:, nsl] tiles for the Q projection."""
                    nsl = slice(nci * 512, (nci + 1) * 512)
                    xts = []
                    for c in range(CCH):
                        t = xs.tile([128, 512], F32R, tag="x", name="xt")
                        nc.sync.dma_start(out=t, in_=xdr[c][:, nsl])
                        xts.append(t)
                    return xts

                def qproj_i(nci, i, xts, qts):
                    """one i-chunk of qT = Wq^T x[:, nsl]."""
                    acc = psA.tile([128, 512], F32, tag="acc", name="qacc")
                    for c in range(CCH):
                        nc.tensor.matmul(
                            acc[:], wq_sb[c][:, i * 128:(i + 1) * 128], xts[c][:],
                            start=(c == 0), stop=(c == CCH - 1))
                    qt = qp.tile([128, 512], F32R, tag="q", name="qt")
                    nc.vector.tensor_copy(qt[:], acc[:])
                    qts.append(qt)

                def qproj(nci):
                    xts = qproj_load(nci)
                    qts = []
                    for i in range(CCH):
                        qproj_i(nci, i, xts, qts)
                    return qts

                qts0 = qproj(0)

                # ---------------- kT = Wk^T ctx^T  [inner, M] ----------------
                prolog_cm = tc.tile_pool(name="prolog", bufs=1)
                prolog = prolog_cm.__enter__()
                wk_sb = [prolog.tile([128, INNER], F32R, tag=f"wk{c}",
                                     name=f"wk{c}") for c in range(XCH)]
                wv_sb = [prolog.tile([128, INNER], F32R, tag=f"wv{c}",
                                     name=f"wv{c}") for c in range(XCH)]
                for c in range(XCH):
                    nc.sync.dma_start(out=wk_sb[c], in_=wkr[c])
                for c in range(XCH):
                    nc.gpsimd.dma_start(out=wv_sb[c], in_=wvr[c])
                for mh in range(2):  # m halves of 512
                    cts = []
                    for c in range(XCH):
                        t = prolog.tile([128, 512], F32R, tag=f"ctx{c}",
                                        name="ctx")
                        nc.gpsimd.dma_start(out=t, in_=ctxr[c][:, mh * 512:(mh + 1) * 512])
                        cts.append(t)
                    for i in range(CCH):
                        acc = psA.tile([128, 512], F32, tag="acc", name="kacc")
                        for c in range(XCH):
                            nc.tensor.matmul(
                                acc[:], wk_sb[c][:, i * 128:(i + 1) * 128], cts[c][:],
                                start=(c == 0), stop=(c == XCH - 1))
                        nc.vector.tensor_copy(
                            kt_sb[i][:, mh * 512:(mh + 1) * 512], acc[:])
                    # ------------ v' = (ctx Wv | 1)  [M, 8*65] ------------
                    for ml in range(4):
                        m = mh * 4 + ml
                        acc = psA.tile([128, 512], F32, tag="acc", name="vacc")
                        for c in range(XCH):
                            nc.tensor.matmul(
                                acc[:], cts[c][:, ml * 128:(ml + 1) * 128], wv_sb[c][:],
                                start=(c == 0), stop=(c == XCH - 1))
                        vdst = v_sb[m].rearrange("p (h j) -> p h j", j=DH + 1)
                        nc.vector.tensor_copy(
                            vdst[:, :, 0:DH],
                            acc.rearrange("p (h j) -> p h j", j=DH))
                        nc.vector.tensor_copy(
                            vdst[:, :, DH:DH + 1],
                            ones_sb.rearrange("p (h j) -> p h j", j=1))

                prolog_cm.__exit__(None, None, None)

                # ---------------- pipeline stages ----------------
                def attn_qk(nci, h, qts):
                    """S^T = kT_h^T q_h, exp -> P^T tiles (3 groups)."""
                    hb = (h % 2) * 64
                    qh = qts[h // 2][hb:hb + 64, :]
                    pts = []
                    for (g0, g1) in SGROUPS:
                        w = (g1 - g0) * 512
                        st = psS.tile([128, 1536], F32, tag="s", name="st")
                        for j, m in enumerate(range(g0, g1)):
                            nc.tensor.matmul(
                                st[:, j * 512:(j + 1) * 512],
                                kt_sb[h // 2][hb:hb + 64, m * 128:(m + 1) * 128],
                                qh, start=True, stop=True)
                        pt = pp.tile([128, 1536], F32R, tag="p", name="pt")
                        nc.scalar.activation(pt[:, :w], st[:, :w], AF.Exp, scale=SCALE)
                        pts.append(pt)
                    return pts

                def attn_av(nci, h, pts, sums_pair):
                    """O' = v'^T P^T accumulated over m; evacuate + denominators."""
                    nsl = slice(nci * 512, (nci + 1) * 512)
                    acc = psA.tile([128, 512], F32, tag="acc", name="avacc")
                    for gi, (g0, g1) in enumerate(SGROUPS):
                        for j, m in enumerate(range(g0, g1)):
                            nc.tensor.matmul(
                                acc[0:DH + 1, :],
                                v_sb[m][:, h * (DH + 1):(h + 1) * (DH + 1)],
                                pts[gi][:, j * 512:(j + 1) * 512],
                                start=(m == 0), stop=(m == MCI - 1))
                    k = h // 2
                    if h % 2 == 0:
                        nc.vector.tensor_copy(o_all[k][0:64, nsl], acc[0:64, :])
                    else:
                        ot = ostp.tile([64, 512], BF16, tag="oev", name="oev")
                        nc.vector.tensor_copy(ot[:], acc[0:64, :])
                        nc.gpsimd.dma_start(out=o_all[k][64:128, nsl], in_=ot[:])
                    stg = misc.tile([128, 512], F32, tag="sstg", name="sstg")
                    nc.vector.tensor_copy(stg[64:65, :], acc[64:65, :])
                    nc.gpsimd.dma_start(out=sums_pair[h:h + 1, :],
                                         in_=stg[64:65, :])

                def nc_recip(nci, sums_nc):
                    """reciprocal of all 8 denominator rows -> one broadcast
                    tile [128, 2048]: partition p, col k*512+j holds the
                    reciprocal for head 2k + (p >= 64)."""
                    r0 = nci * 8
                    rec8 = sump.tile([8, 512], F32, tag="rec", name="rec",
                                     bufs=1)
                    recb8 = sump.tile([8, 512], BF16, tag="recb", name="recb",
                                      bufs=1)
                    nc.vector.reciprocal_approx_fast(out=rec8[:], in_=sums_nc[:])
                    nc.vector.tensor_copy(recb8[:], rec8[:])
                    nc.gpsimd.dma_start(out=rscr_d[r0:r0 + 8, :], in_=recb8[:])
                    rr = rrp.tile([128, 2048], BF16, tag="rrep", name="rr")
                    for par in range(2):
                        src = bass.AP(
                            tensor=rscr_d[:].tensor,
                            offset=rscr_d[r0 + par:r0 + par + 1, :].offset,
                            ap=[[0, 64], [1024, 4], [1, 512]])
                        nc.gpsimd.dma_start(out=rr[par * 64:(par + 1) * 64, :],
                                            in_=src)
                    return rr

                def nc_normalize(nci, rr):
                    nsl = slice(nci * 512, (nci + 1) * 512)
                    for k in range(CCH):
                        osl = o_all[k][:, nsl]
                        nc.vector.tensor_mul(osl, osl,
                                             rr[:, k * 512:(k + 1) * 512])

                def oproj_c(nci, c):
                    """one c-chunk of out = Wo^T O_norm + bo + x."""
                    nsl = slice(nci * 512, (nci + 1) * 512)
                    xrt = xrp.tile([128, 512], F32, tag="xres", name="xres")
                    nc.sync.dma_start(out=xrt, in_=xdr[c][:, nsl].bitcast(F32))
                    acc = psA.tile([128, 512], F32, tag="acc", name="oacc")
                    for k in range(CCH):
                        nc.tensor.matmul(
                            acc[:], wo_bf[k][:, c * 128:(c + 1) * 128],
                            o_all[k][:, nsl],
                            start=(k == 0), stop=(k == CCH - 1))
                    ott = outp.tile([128, 512], F32, tag="out", name="ott")
                    nc.vector.scalar_tensor_tensor(
                        out=ott[:], in0=acc[:], scalar=bo_sb[:, c:c + 1],
                        in1=xrt[:], op0=ALU.add, op1=ALU.add)
                    nc.sync.dma_start(
                        out=out_d[c * 128:(c + 1) * 128, nsl], in_=ott[:])

                # ---------------- main software pipeline ----------------
                steps = [(nci, h) for nci in range(NCI) for h in range(HEADS)]
                qts_by_nc = {0: qts0}
                sums_tiles = {}
                prev = None        # (nci, h, pts)
                norm_q = []        # [(nci, rr)] awaiting the 4 TT-normalizes
                fill_q = []        # FIFO of deferred 4-MM work items

                def drain_one_norm():
                    qnci, qrr = norm_q.pop(0)
                    nc_normalize(qnci, qrr)
                    for c in range(CCH):
                        fill_q.append(("oproj", qnci, c, None, None))

                def run_fill(item):
                    kind, a, b, xts, qts = item
                    if kind == "qproj":
                        qproj_i(a, b, xts, qts)
                    else:
                        oproj_c(a, b)

                for (nci, h) in steps:
                    if h == 0:
                        sums_tiles[nci] = sump.tile([8, 512], F32, tag="sums",
                                                    name="sums", bufs=2)
                    if h == 0 and nci < NCI - 1:
                        xts_n = qproj_load(nci + 1)
                        qts_n = []
                        qts_by_nc[nci + 1] = qts_n
                        for i in range(CCH):
                            fill_q.append(("qproj", nci + 1, i, xts_n, qts_n))
                    pts = attn_qk(nci, h, qts_by_nc[nci])
                    if prev is not None:
                        pnci, ph, ppts = prev
                        attn_av(pnci, ph, ppts, sums_tiles[pnci])
                        if ph == HEADS - 1:
                            norm_q.append((pnci, nc_recip(pnci,
                                                          sums_tiles[pnci])))
                    if norm_q and not (prev is not None and prev[1] == HEADS - 1):
                        drain_one_norm()
                    if fill_q:
                        run_fill(fill_q.pop(0))
                    prev = (nci, h, pts)
                # drain pipeline
                pnci, ph, ppts = prev
                attn_av(pnci, ph, ppts, sums_tiles[pnci])
                norm_q.append((pnci, nc_recip(pnci, sums_tiles[pnci])))
                while norm_q:
                    drain_one_norm()
                while fill_q:
                    run_fill(fill_q.pop(0))

    nc.compile()
    return nc


def kernel(x, context, Wq, Wk, Wv, Wo, bo):
    global _PROG
    if _PROG is None:
        _PROG = _build()
    nc = _PROG

    x = np.asarray(x, np.float32).reshape(B, C, N)
    ctxT = np.ascontiguousarray(
        np.asarray(context, np.float32).transpose(0, 2, 1))
    wq = np.ascontiguousarray(np.asarray(Wq, np.float32))
    wk = np.ascontiguousarray(np.asarray(Wk, np.float32))
    wv = np.ascontiguousarray(np.asarray(Wv, np.float32))
    wo = np.ascontiguousarray(np.asarray(Wo, np.float32))
    bov = np.ascontiguousarray(np.asarray(bo, np.float32))

    in_maps = [
        {"x": np.ascontiguousarray(x[b]), "ctxT": ctxT[b],
         "wq": wq, "wk": wk, "wv": wv, "wo": wo, "bo": bov}
        for b in range(B)
    ]
    res = run_bass_kernel_spmd(nc, in_maps, core_ids=list(range(8)))
    out = np.stack([res.results[b]["out"] for b in range(B)], axis=0)
    return out.reshape(B, C, HH, WW).astype(np.float32)

